# revision 1
# baseline (speedup 1.0000x reference)
"""Trainium2 Bass kernel for nn_EventEmbeddingModel (segment_reduce).

out[b] = (sum_{l < hist_len[b]} emb[history[b, l]]  or  emb[entities[b]] if
hist_len[b] == 0) @ W.T + bias

Strategy (8 NeuronCores, data-parallel over batch):
- Host: fold the hist_len==0 fallback into slot 0, sort rows by effective
  history length (desc), deal rows round-robin to cores so all cores share an
  identical per-tile max-L schedule; pad short rows with an appended zero row.
- Device (per core, 16 tiles of 128 rows): L_t indirect row-gathers from the
  (replicated) embedding table into SBUF, vector reduce over L, PE transpose +
  matmul with W.T + bias add, DMA out.
The gather is descriptor-rate bound, so the host-side compaction (variable
L_t instead of dense L=50) cuts gathered rows ~2x.
"""
import os
import sys

if "/opt/trn_rl_repo" not in sys.path:
    sys.path.insert(0, "/opt/trn_rl_repo")

import numpy as np

B, L, V, D = 16384, 50, 1000000, 128
NCORES = 8
BC = B // NCORES          # 2048 rows per core
P = 128                   # partition dim / tile rows
NT = BC // P              # 16 tiles per core

LAST_RESULTS = None       # test harness reads exec_time_ns from here

_BUILD_CACHE = {}


def _maybe_install_ntff_shim():
    """Register the axon NTFF profile hook so BASS_TRACE=1 yields exec_time_ns."""
    import types
    import ctypes
    import contextlib

    if "antenv.axon_hooks" in sys.modules:
        return
    so_path = "/opt/axon/libaxon_pjrt.so"
    if not os.path.exists(so_path):
        return
    try:
        lib = ctypes.CDLL(so_path)
        if not hasattr(lib, "axon_start_nrt_profile"):
            return
        lib.axon_start_nrt_profile.argtypes = [
            ctypes.POINTER(ctypes.c_int64),
            ctypes.c_size_t,
        ]
        lib.axon_start_nrt_profile.restype = ctypes.c_int64
        lib.axon_stop_nrt_profile.argtypes = [ctypes.c_char_p]
        lib.axon_stop_nrt_profile.restype = ctypes.c_int64

        @contextlib.contextmanager
        def _hook(output_dir, device_ids):
            import jax
            jax.devices()
            if device_ids:
                ids = (ctypes.c_int64 * len(device_ids))(*device_ids)
                rc = lib.axon_start_nrt_profile(ids, len(device_ids))
            else:
                rc = lib.axon_start_nrt_profile(None, 0)
            if rc != 0:
                raise RuntimeError(f"axon_start_nrt_profile rc={rc}")
            try:
                yield
            finally:
                n = lib.axon_stop_nrt_profile(str(output_dir).encode())
                if n <= 0:
                    print(f"ntff profile: {n} files written", file=sys.stderr)

        mod = types.ModuleType("antenv.axon_hooks")
        mod.get_axon_ntff_profile_hook = lambda: _hook
        sys.modules["antenv.axon_hooks"] = mod
    except Exception:
        pass


def _build(tile_ls):
    """Build + compile the per-core Bass program for a tuple of per-tile Ls."""
    from concourse import bass, bacc, mybir, tile

    key = tuple(int(x) for x in tile_ls)
    if key in _BUILD_CACHE:
        return _BUILD_CACHE[key]

    f32 = mybir.dt.float32
    i32 = mybir.dt.int32

    nc = bacc.Bacc("TRN2", target_bir_lowering=False, debug=False)
    table = nc.declare_dram_parameter("table", [V + 1, D], f32, isOutput=False)
    ident = nc.declare_dram_parameter("ident", [P, P], f32, isOutput=False)
    idx = nc.declare_dram_parameter("idx", [BC, L], i32, isOutput=False)
    wt = nc.declare_dram_parameter("wt", [D, D], f32, isOutput=False)
    bias_bc = nc.declare_dram_parameter("bias_bc", [P, D], f32, isOutput=False)
    out = nc.declare_dram_parameter("out", [BC, D], f32, isOutput=True)

    with tile.TileContext(nc) as tc:
        with tc.tile_pool(name="const", bufs=1) as const, \
             tc.tile_pool(name="work", bufs=4) as work, \
             tc.tile_pool(name="psum", bufs=2, space="PSUM") as psum:
            # tile-0 indices load first (tiny) so gathers start immediately;
            # the rest streams in behind it
            idx0 = const.tile([P, L], i32)
            nc.sync.dma_start(out=idx0[:], in_=idx[0:P, :])
            idx_rest = const.tile([P, NT - 1, L], i32)
            nc.sync.dma_start(
                out=idx_rest[:],
                in_=idx[P:].rearrange("(t p) l -> p t l", t=NT - 1, p=P),
            )
            identity = const.tile([P, P], f32)
            nc.sync.dma_start(out=identity[:], in_=ident[:])
            wt_t = const.tile([D, D], f32)
            nc.sync.dma_start(out=wt_t[:], in_=wt[:])
            bias_t = const.tile([P, D], f32)
            nc.sync.dma_start(out=bias_t[:], in_=bias_bc[:])

            for t, lt in enumerate(key):
                lt = max(1, int(lt))
                rows = slice(t * P, (t + 1) * P)
                g = work.tile([P, lt * D], f32, tag="g", name=f"g_{t}")
                for l in range(lt):
                    nc.gpsimd.indirect_dma_start(
                        out=g[:, l * D:(l + 1) * D],
                        out_offset=None,
                        in_=table[:],
                        in_offset=bass.IndirectOffsetOnAxis(
                            ap=(idx0[:, l:l + 1] if t == 0
                                else idx_rest[:, t - 1, l:l + 1]),
                            axis=0,
                        ),
                    )
                acc = work.tile([P, D], f32, tag="acc")
                nc.vector.tensor_reduce(
                    out=acc[:],
                    in_=g[:].rearrange("p (l d) -> p d l", l=lt, d=D),
                    axis=mybir.AxisListType.X,
                    op=mybir.AluOpType.add,
                )
                acc_t_ps = psum.tile([P, D], f32, tag="tps")
                nc.tensor.transpose(out=acc_t_ps[:], in_=acc[:], identity=identity[:])
                acc_t = work.tile([P, D], f32, tag="accT")
                nc.vector.tensor_copy(out=acc_t[:], in_=acc_t_ps[:])
                out_ps = psum.tile([P, D], f32, tag="ops")
                nc.tensor.matmul(
                    out=out_ps[:], lhsT=acc_t[:], rhs=wt_t[:], start=True, stop=True
                )
                out_sb = work.tile([P, D], f32, tag="out")
                nc.vector.tensor_tensor(
                    out=out_sb[:], in0=out_ps[:], in1=bias_t[:],
                    op=mybir.AluOpType.add,
                )
                nc.sync.dma_start(out=out[rows, :], in_=out_sb[:])
    nc.compile()
    _BUILD_CACHE[key] = nc
    return nc


def _prepare(entities, history, hist_len):
    """Host-side index prep. Returns (per-core idx arrays int32 [BC, L],
    per-tile Ls, scatter positions [BC, NCORES])."""
    ent = np.asarray(entities).astype(np.int64)
    hist = np.asarray(history).astype(np.int64).copy()
    hl = np.asarray(hist_len).astype(np.int64)

    empty = hl == 0
    hist[empty, 0] = ent[empty]
    hl_eff = np.maximum(hl, 1)

    order = np.argsort(-hl_eff, kind="stable")       # desc by effective length
    hl_sorted = hl_eff[order]

    # positions[j, c] = original row index handled by core c at local row j
    positions = order.reshape(BC, NCORES)
    hl_pos = hl_sorted.reshape(BC, NCORES)

    # per-tile L = max over the 8*128-row window = first element (desc sorted)
    tile_ls = [int(hl_sorted[t * P * NCORES]) for t in range(NT)]

    # build padded int32 index arrays per core
    col = np.arange(L)[None, :]
    idx_cores = []
    for c in range(NCORES):
        rows = positions[:, c]
        h = hist[rows]                                # [BC, L]
        valid = col < hl_pos[:, c][:, None]           # [BC, L]
        hi = np.where(valid, h, V).astype(np.int32)
        idx_cores.append(np.ascontiguousarray(hi))
    return idx_cores, tile_ls, positions


def kernel(entities, history, hist_len, entities_emb, W, b):
    global LAST_RESULTS
    from concourse.bass_utils import run_bass_kernel_spmd

    if os.environ.get("BASS_TRACE"):
        _maybe_install_ntff_shim()

    idx_cores, tile_ls, positions = _prepare(entities, history, hist_len)

    emb = np.asarray(entities_emb, dtype=np.float32)
    table = np.empty((V + 1, D), dtype=np.float32)
    table[:V] = emb
    table[V] = 0.0
    wt = np.ascontiguousarray(np.asarray(W, dtype=np.float32).T)
    bias_bc = np.tile(np.asarray(b, dtype=np.float32)[None, :], (P, 1))
    ident_np = np.eye(P, dtype=np.float32)

    nc = _build(tile_ls)
    in_maps = [
        {"table": table, "idx": idx_cores[c], "wt": wt, "bias_bc": bias_bc,
         "ident": ident_np}
        for c in range(NCORES)
    ]
    res = run_bass_kernel_spmd(nc, in_maps, list(range(NCORES)))
    LAST_RESULTS = res

    out = np.empty((B, D), dtype=np.float32)
    for c in range(NCORES):
        out[positions[:, c]] = res.results[c]["out"]
    return out

